# revision 9
# baseline (speedup 1.0000x reference)
"""Multi-head causal attention (scores = K @ Q^T variant) on 8 TRN2 NeuronCores.

Head-parallel sharding: core c computes heads (2c, 2c+1) end-to-end and the
host concatenates the per-core [T, 128] outputs along the feature axis.

Per-core kernel layout notes:
  - Host passes x transposed ([D, T]) so every projection matmul has the
    contraction dim (d) on SBUF partitions with zero on-device transposes.
  - Q^T/K^T are stored [128, T] with head0 on partitions 0-63 and head1 on
    64-127, letting the S^T matmuls for both heads run concurrently on
    disjoint PE row-groups (tile_position).
  - Scores are computed transposed (S^T[j, i] = Q_j . K_i) so that the AV
    contraction (over j) lands on the partition axis with no transposes.
  - V is stored in natural layout with a fused ones-column ([V | 1]) so a
    single AV matmul produces both the weighted sum and the softmax
    denominator (PSUM row 64).
  - Softmax skips the max-subtraction (scores are ~N(0,1); exp is safe in
    fp32) which matches jax.nn.softmax to fp32 rounding.
  - All big matmuls use float32r (full PE rate); transposes use strict fp32.
"""

import numpy as np

T, D, H, HS = 4096, 1024, 16, 64
NCORES = 8
HPC = H // NCORES  # heads per core = 2
DC = D // 128      # 8 contraction chunks
TC = T // 512      # 8 t-chunks for projections
IB = T // 512      # 8 i-blocks (512 output rows each)
JBN = T // 128     # 32 j-blocks (128 keys each)

_cached_nc = None


def _emit(tc, nc, xT, w6, out):
    import concourse.bass as bass  # noqa: F401
    import concourse.mybir as mybir

    f32 = mybir.dt.float32
    f32r = mybir.dt.float32r
    Exp = mybir.ActivationFunctionType.Exp
    ne = mybir.AluOpType.not_equal
    ge = mybir.AluOpType.is_ge

    def r_(ap):
        return ap.bitcast(f32r)

    with (
        tc.tile_pool(name="const", bufs=1) as constp,
        tc.tile_pool(name="wpool", bufs=1) as wpool,
        tc.tile_pool(name="bigp", bufs=1) as bigp,
    ):
        # ---- constants -------------------------------------------------
        # id64: two stacked 64x64 identities so both head slices (partition
        # offset 0 and 64) see an identity for the V transposes.
        id64 = constp.tile([128, 64], f32)
        nc.gpsimd.memset(id64, 0.0)
        nc.gpsimd.affine_select(
            out=id64, in_=id64, compare_op=ne, fill=1.0,
            base=0, channel_multiplier=1, pattern=[[-1, 64]],
        )
        nc.gpsimd.affine_select(
            out=id64, in_=id64, compare_op=ne, fill=1.0,
            base=-64, channel_multiplier=1, pattern=[[-1, 64]],
        )
        # id65: identity for the [65, 128] -> [128, 65] output transposes.
        id65 = constp.tile([128, 65], f32)
        nc.gpsimd.memset(id65, 0.0)
        nc.gpsimd.affine_select(
            out=id65, in_=id65, compare_op=ne, fill=1.0,
            base=0, channel_multiplier=1, pattern=[[-1, 65]],
        )
        # Causal masks for the 4 diagonal block offsets: keep iff il >= jl + 128*q.
        mask4 = constp.tile([128, 4, 512], f32)
        for q in range(4):
            nc.gpsimd.memset(mask4[:, q, :], 1.0)
            nc.gpsimd.affine_select(
                out=mask4[:, q, :], in_=mask4[:, q, :], compare_op=ge, fill=0.0,
                base=-128 * q, channel_multiplier=-1, pattern=[[1, 512]],
            )

        # ---- weights ---------------------------------------------------
        w6sb = wpool.tile([128, DC, 6 * HS], f32r)
        for dc in range(DC):
            nc.sync.dma_start(out=w6sb[:, dc, :], in_=r_(w6[dc * 128:(dc + 1) * 128, :]))

        # ---- persistent activations -----------------------------------
        QT = bigp.tile([128, T], f32r)   # head0 rows 0-63, head1 rows 64-127
        KT = bigp.tile([128, T], f32r)
        Vext0 = bigp.tile([128, JBN, HS + 1], f32r)
        Vext1 = bigp.tile([128, JBN, HS + 1], f32r)
        onesb = constp.tile([128, JBN], f32)
        nc.gpsimd.memset(onesb, 1.0)
        nc.vector.tensor_copy(Vext0[:, :, HS], onesb)
        nc.vector.tensor_copy(Vext1[:, :, HS], onesb)

        # ---- phase 1: projections -------------------------------------
        with (
            tc.tile_pool(name="xpool", bufs=2) as xpool,
            tc.tile_pool(name="vtp", bufs=2) as vtp,
            tc.tile_pool(name="pp", bufs=4, space="PSUM") as pp,
            tc.tile_pool(name="ptv", bufs=2, space="PSUM") as ptvp,
        ):
            for tcj in range(TC):
                ts = slice(tcj * 512, (tcj + 1) * 512)
                xt = xpool.tile([128, DC, 512], f32r, tag="xt")
                for dc in range(DC):
                    nc.sync.dma_start(out=xt[:, dc, :], in_=r_(xT[dc * 128:(dc + 1) * 128, ts]))
                for fc, dest in ((0, QT), (1, KT)):
                    ps = pp.tile([128, 512], f32, tag="proj")
                    for dc in range(DC):
                        nc.tensor.matmul(
                            ps,
                            lhsT=w6sb[:, dc, fc * 128:(fc + 1) * 128],
                            rhs=xt[:, dc, :],
                            start=(dc == 0), stop=(dc == DC - 1),
                        )
                    nc.scalar.copy(dest[:, ts], ps)
                psv = pp.tile([128, 512], f32, tag="proj")
                for dc in range(DC):
                    nc.tensor.matmul(
                        psv,
                        lhsT=w6sb[:, dc, 256:384],
                        rhs=xt[:, dc, :],
                        start=(dc == 0), stop=(dc == DC - 1),
                    )
                vts = vtp.tile([128, 512], f32, tag="vts")
                nc.scalar.copy(vts, psv)
                for h in range(HPC):
                    vdst = Vext0 if h == 0 else Vext1
                    for q in range(4):
                        ptv = ptvp.tile([128, 64], f32, tag="tv")
                        nc.tensor.transpose(
                            ptv,
                            in_=vts[h * 64:(h + 1) * 64, q * 128:(q + 1) * 128],
                            identity=id64[h * 64:(h + 1) * 64, :],
                            tile_position=(h * 64, 0),
                        )
                        nc.vector.tensor_copy(vdst[:, tcj * 4 + q, 0:HS], ptv)

        # ---- phase 2: attention ---------------------------------------
        with (
            tc.tile_pool(name="esp", bufs=3) as esp,
            tc.tile_pool(name="osbp", bufs=2) as osbp,
            tc.tile_pool(name="finp", bufs=4) as finp,
            tc.tile_pool(name="sp", bufs=2, space="PSUM") as sp,
            tc.tile_pool(name="op", bufs=2, space="PSUM") as op,
            tc.tile_pool(name="tp", bufs=2, space="PSUM") as tp,
        ):
            for ib in range(IB):
                isl = slice(ib * 512, (ib + 1) * 512)
                njb = 4 * (ib + 1)
                po = [op.tile([65, 512], f32, tag="o", name=f"po{h}_{ib}") for h in range(HPC)]
                pending_av = None
                for jb in range(njb):
                    ps = sp.tile([128, 2, 512], f32, tag="s")
                    for h in range(HPC):
                        nc.tensor.matmul(
                            ps[:, h, :],
                            lhsT=QT[h * 64:(h + 1) * 64, jb * 128:(jb + 1) * 128],
                            rhs=KT[h * 64:(h + 1) * 64, isl],
                            start=True, stop=True,
                            tile_position=(h * 64, 0),
                        )
                    es = esp.tile([128, 2, 512], f32r, tag="es")
                    nc.scalar.activation(es, ps, Exp, scale=float(1.0 / np.sqrt(HS)))
                    q = jb - 4 * ib
                    if q >= 0:  # diagonal block: zero out j > i entries
                        for h in range(HPC):
                            nc.vector.tensor_mul(es[:, h, :], es[:, h, :], r_(mask4[:, q, :]))
                    if pending_av is not None:
                        pjb, pes = pending_av
                        for h in range(HPC):
                            nc.tensor.matmul(
                                po[h],
                                lhsT=Vext0[:, pjb, :] if h == 0 else Vext1[:, pjb, :],
                                rhs=pes[:, h, :],
                                start=(pjb == 0), stop=False,
                            )
                    pending_av = (jb, es)
                pjb, pes = pending_av
                for h in range(HPC):
                    nc.tensor.matmul(
                        po[h],
                        lhsT=Vext0[:, pjb, :] if h == 0 else Vext1[:, pjb, :],
                        rhs=pes[:, h, :],
                        start=(pjb == 0), stop=True,
                    )
                # finalize: transpose O^T, divide by denominator, store
                for h in range(HPC):
                    osb = osbp.tile([65, 512], f32, tag="osb")
                    nc.vector.tensor_copy(osb, po[h])
                    for qq in range(4):
                        pt = tp.tile([128, 65], f32, tag="tp")
                        nc.tensor.transpose(
                            pt,
                            in_=osb[:, qq * 128:(qq + 1) * 128],
                            identity=id65[0:65, :],
                        )
                        rr = finp.tile([128, 1], f32, tag="rr")
                        nc.vector.reciprocal(rr, pt[:, 64:65])
                        ob = finp.tile([128, 64], f32, tag="ob")
                        nc.vector.tensor_scalar_mul(ob, pt[:, 0:64], rr)
                        r0 = ib * 512 + qq * 128
                        nc.sync.dma_start(
                            out=out[r0:r0 + 128, h * 64:(h + 1) * 64], in_=ob
                        )


# walrus engine-instruction encodings have a single sync-wait slot; hoist
# extra waits onto per-wait NoOps for everything except drains (which expand
# into their own wait sequences).
_NO_HOIST_TYPES = frozenset({"InstNoOp"})


def _split_matmul_waits(nc):
    """Hoist multi-waits off Matmult instructions onto preceding PE NoOps.

    4-byte (fp32/fp32r) matmuls lower through walrus's S3_LW struct which
    only has room for a single sync-wait command; Tile freely attaches
    several. Waits execute on the engine's sequencer in program order, so
    moving them to an immediately-preceding NoOp is semantics-preserving.
    """
    import bass_rust

    for f in nc.m.functions:
        for blk in f.blocks:
            out = []
            changed = False
            for inst in blk.instructions:
                si = getattr(inst, "sync_info", None)
                if (
                    type(inst).__name__ not in _NO_HOIST_TYPES
                    and si is not None
                    and len(si.on_wait) >= 2
                ):
                    waits = list(si.on_wait)
                    for k, w in enumerate(waits[:-1]):
                        nop = bass_rust.InstNoOp(name=f"{inst.name}_hoistw{k}")
                        nop.engine = inst.engine
                        nop.sync_info = bass_rust.SyncInfo(
                            on_wait=[w], on_update=[]
                        )
                        out.append(nop)
                    si.on_wait = [waits[-1]]
                    changed = True
                out.append(inst)
            if changed:
                blk.instructions = out


def _build_program():
    import concourse.bass as bass
    import concourse.mybir as mybir
    import concourse.tile as tile

    nc = bass.Bass("TRN2", target_bir_lowering=False, debug=False, num_devices=NCORES)
    xT = nc.dram_tensor("xT", [D, T], mybir.dt.float32, kind="ExternalInput").ap()
    w6 = nc.dram_tensor("w6", [D, 6 * HS], mybir.dt.float32, kind="ExternalInput").ap()
    out = nc.dram_tensor("out", [T, HPC * HS], mybir.dt.float32, kind="ExternalOutput").ap()

    with tile.TileContext(nc) as tc:
        _emit(tc, nc, xT, w6, out)
    _split_matmul_waits(nc)
    return nc


def _in_maps(x, Wk, Wq, Wv):
    xTh = np.ascontiguousarray(np.asarray(x, dtype=np.float32).T)
    maps = []
    for c in range(NCORES):
        h0, h1 = HPC * c, HPC * c + 1
        W6 = np.concatenate(
            [Wq[h0], Wq[h1], Wk[h0], Wk[h1], Wv[h0], Wv[h1]], axis=1
        ).astype(np.float32)
        maps.append({"xT": xTh, "w6": np.ascontiguousarray(W6)})
    return maps


def get_program():
    global _cached_nc
    if _cached_nc is None:
        _cached_nc = _build_program()
    return _cached_nc


def kernel(x, Wk, Wq, Wv):
    from concourse.bass_utils import run_bass_kernel_spmd

    nc = get_program()
    res = run_bass_kernel_spmd(nc, _in_maps(x, Wk, Wq, Wv), core_ids=list(range(NCORES)))
    outs = [res.results[c]["out"] for c in range(NCORES)]
    return np.concatenate(outs, axis=1)


# revision 11
# speedup vs baseline: 1.1462x; 1.1462x over previous
"""Multi-head causal attention (scores = K @ Q^T variant) on 8 TRN2 NeuronCores.

Head-parallel sharding: core c computes heads (2c, 2c+1) end-to-end and the
host concatenates the per-core [T, 128] outputs along the feature axis.

Per-core kernel layout notes:
  - Host passes x transposed ([D, T]) so every projection matmul has the
    contraction dim (d) on SBUF partitions with zero on-device transposes.
  - Q^T/K^T are stored [128, T] with head0 on partitions 0-63 and head1 on
    64-127, letting the S^T matmuls for both heads run concurrently on
    disjoint PE row-groups (tile_position).
  - Scores are computed transposed (S^T[j, i] = Q_j . K_i) so that the AV
    contraction (over j) lands on the partition axis with no transposes.
  - V is stored in natural layout with a fused ones-column ([V | 1]) so a
    single AV matmul produces both the weighted sum and the softmax
    denominator (PSUM row 64).
  - Softmax skips the max-subtraction (scores are ~N(0,1); exp is safe in
    fp32) which matches jax.nn.softmax to fp32 rounding.
  - All big matmuls use float32r (full PE rate); transposes use strict fp32.
"""

import numpy as np

T, D, H, HS = 4096, 1024, 16, 64
NCORES = 8
HPC = H // NCORES  # heads per core = 2
DC = D // 128      # 8 contraction chunks
TC = T // 512      # 8 t-chunks for projections
IB = T // 512      # 8 i-blocks (512 output rows each)
JBN = T // 128     # 32 j-blocks (128 keys each)

_cached_nc = None


def _emit(tc, nc, xT, w6, out):
    import concourse.bass as bass  # noqa: F401
    import concourse.mybir as mybir

    f32 = mybir.dt.float32
    bf16 = mybir.dt.bfloat16
    Exp = mybir.ActivationFunctionType.Exp
    ne = mybir.AluOpType.not_equal
    ge = mybir.AluOpType.is_ge

    with (
        tc.tile_pool(name="const", bufs=1) as constp,
        tc.tile_pool(name="wpool", bufs=1) as wpool,
        tc.tile_pool(name="bigp", bufs=1) as bigp,
    ):
        # ---- constants -------------------------------------------------
        # id64: two stacked 64x64 identities so both head slices (partition
        # offset 0 and 64) see an identity for the V transposes.
        id64 = constp.tile([128, 64], bf16)
        nc.gpsimd.memset(id64, 0.0)
        nc.gpsimd.affine_select(
            out=id64, in_=id64, compare_op=ne, fill=1.0,
            base=0, channel_multiplier=1, pattern=[[-1, 64]],
        )
        nc.gpsimd.affine_select(
            out=id64, in_=id64, compare_op=ne, fill=1.0,
            base=-64, channel_multiplier=1, pattern=[[-1, 64]],
        )
        # id65: identity for the [65, 128] -> [128, 65] output transposes.
        id65 = constp.tile([128, 65], f32)
        nc.gpsimd.memset(id65, 0.0)
        nc.gpsimd.affine_select(
            out=id65, in_=id65, compare_op=ne, fill=1.0,
            base=0, channel_multiplier=1, pattern=[[-1, 65]],
        )
        # Causal masks for the 4 diagonal block offsets: keep iff il >= jl + 128*q.
        mask4 = constp.tile([128, 4, 512], bf16)
        for q in range(4):
            nc.gpsimd.memset(mask4[:, q, :], 1.0)
            nc.gpsimd.affine_select(
                out=mask4[:, q, :], in_=mask4[:, q, :], compare_op=ge, fill=0.0,
                base=-128 * q, channel_multiplier=-1, pattern=[[1, 512]],
            )

        # ---- weights ---------------------------------------------------
        w6sb = wpool.tile([128, DC, 6 * HS], bf16)
        for dc in range(DC):
            nc.sync.dma_start(out=w6sb[:, dc, :], in_=w6[dc * 128:(dc + 1) * 128, :])

        # ---- persistent activations -----------------------------------
        QT = bigp.tile([128, T], bf16)   # head0 rows 0-63, head1 rows 64-127
        KT = bigp.tile([128, T], bf16)
        Vext0 = bigp.tile([128, JBN, HS + 1], bf16)
        Vext1 = bigp.tile([128, JBN, HS + 1], bf16)
        onesb = constp.tile([128, JBN], bf16)
        nc.gpsimd.memset(onesb, 1.0)
        nc.vector.tensor_copy(Vext0[:, :, HS], onesb)
        nc.vector.tensor_copy(Vext1[:, :, HS], onesb)

        # ---- phase 1: projections -------------------------------------
        with (
            tc.tile_pool(name="xpool", bufs=2) as xpool,
            tc.tile_pool(name="vtp", bufs=2) as vtp,
            tc.tile_pool(name="pp", bufs=4, space="PSUM") as pp,
            tc.tile_pool(name="ptv", bufs=2, space="PSUM") as ptvp,
        ):
            for tcj in range(TC):
                ts = slice(tcj * 512, (tcj + 1) * 512)
                xt = xpool.tile([128, DC, 512], bf16, tag="xt")
                for dc in range(DC):
                    nc.sync.dma_start(out=xt[:, dc, :], in_=xT[dc * 128:(dc + 1) * 128, ts])
                for fc, dest in ((0, QT), (1, KT)):
                    ps = pp.tile([128, 512], f32, tag="proj")
                    for dc in range(DC):
                        nc.tensor.matmul(
                            ps,
                            lhsT=w6sb[:, dc, fc * 128:(fc + 1) * 128],
                            rhs=xt[:, dc, :],
                            start=(dc == 0), stop=(dc == DC - 1),
                        )
                    nc.scalar.copy(dest[:, ts], ps)
                psv = pp.tile([128, 512], f32, tag="proj")
                for dc in range(DC):
                    nc.tensor.matmul(
                        psv,
                        lhsT=w6sb[:, dc, 256:384],
                        rhs=xt[:, dc, :],
                        start=(dc == 0), stop=(dc == DC - 1),
                    )
                vts = vtp.tile([128, 512], bf16, tag="vts")
                nc.scalar.copy(vts, psv)
                for h in range(HPC):
                    vdst = Vext0 if h == 0 else Vext1
                    for q in range(4):
                        ptv = ptvp.tile([128, 64], bf16, tag="tv")
                        nc.tensor.transpose(
                            ptv,
                            in_=vts[h * 64:(h + 1) * 64, q * 128:(q + 1) * 128],
                            identity=id64[h * 64:(h + 1) * 64, :],
                            tile_position=(h * 64, 0),
                        )
                        nc.vector.tensor_copy(vdst[:, tcj * 4 + q, 0:HS], ptv)

        # ---- phase 2: attention ---------------------------------------
        with (
            tc.tile_pool(name="esp", bufs=3) as esp,
            tc.tile_pool(name="osbp", bufs=2) as osbp,
            tc.tile_pool(name="finp", bufs=4) as finp,
            tc.tile_pool(name="sp", bufs=2, space="PSUM") as sp,
            tc.tile_pool(name="op", bufs=2, space="PSUM") as op,
            tc.tile_pool(name="tp", bufs=2, space="PSUM") as tp,
        ):
            for ib in range(IB):
                isl = slice(ib * 512, (ib + 1) * 512)
                njb = 4 * (ib + 1)
                po = [op.tile([65, 512], f32, tag="o", name=f"po{h}_{ib}") for h in range(HPC)]
                pending_av = None
                for jb in range(njb):
                    ps = sp.tile([128, 2, 512], f32, tag="s")
                    for h in range(HPC):
                        nc.tensor.matmul(
                            ps[:, h, :],
                            lhsT=QT[h * 64:(h + 1) * 64, jb * 128:(jb + 1) * 128],
                            rhs=KT[h * 64:(h + 1) * 64, isl],
                            start=True, stop=True,
                            tile_position=(h * 64, 0),
                        )
                    es = esp.tile([128, 2, 512], bf16, tag="es")
                    nc.scalar.activation(es, ps, Exp, scale=float(1.0 / np.sqrt(HS)))
                    q = jb - 4 * ib
                    if q >= 0:  # diagonal block: zero out j > i entries
                        for h in range(HPC):
                            nc.vector.tensor_mul(es[:, h, :], es[:, h, :], mask4[:, q, :])
                    if pending_av is not None:
                        pjb, pes = pending_av
                        for h in range(HPC):
                            nc.tensor.matmul(
                                po[h],
                                lhsT=Vext0[:, pjb, :] if h == 0 else Vext1[:, pjb, :],
                                rhs=pes[:, h, :],
                                start=(pjb == 0), stop=False,
                            )
                    pending_av = (jb, es)
                pjb, pes = pending_av
                for h in range(HPC):
                    nc.tensor.matmul(
                        po[h],
                        lhsT=Vext0[:, pjb, :] if h == 0 else Vext1[:, pjb, :],
                        rhs=pes[:, h, :],
                        start=(pjb == 0), stop=True,
                    )
                # finalize: transpose O^T, divide by denominator, store
                for h in range(HPC):
                    osb = osbp.tile([65, 512], f32, tag="osb")
                    nc.vector.tensor_copy(osb, po[h])
                    for qq in range(4):
                        pt = tp.tile([128, 65], f32, tag="tp")
                        nc.tensor.transpose(
                            pt,
                            in_=osb[:, qq * 128:(qq + 1) * 128],
                            identity=id65[0:65, :],
                        )
                        rr = finp.tile([128, 1], f32, tag="rr")
                        nc.vector.reciprocal(rr, pt[:, 64:65])
                        ob = finp.tile([128, 64], f32, tag="ob")
                        nc.vector.tensor_scalar_mul(ob, pt[:, 0:64], rr)
                        r0 = ib * 512 + qq * 128
                        nc.sync.dma_start(
                            out=out[r0:r0 + 128, h * 64:(h + 1) * 64], in_=ob
                        )


# walrus engine-instruction encodings have a single sync-wait slot; hoist
# extra waits onto per-wait NoOps for everything except drains (which expand
# into their own wait sequences).
_NO_HOIST_TYPES = frozenset({"InstNoOp"})


def _split_matmul_waits(nc):
    """Hoist multi-waits off Matmult instructions onto preceding PE NoOps.

    4-byte (fp32/fp32r) matmuls lower through walrus's S3_LW struct which
    only has room for a single sync-wait command; Tile freely attaches
    several. Waits execute on the engine's sequencer in program order, so
    moving them to an immediately-preceding NoOp is semantics-preserving.
    """
    import bass_rust

    for f in nc.m.functions:
        for blk in f.blocks:
            out = []
            changed = False
            for inst in blk.instructions:
                si = getattr(inst, "sync_info", None)
                if (
                    type(inst).__name__ not in _NO_HOIST_TYPES
                    and si is not None
                    and len(si.on_wait) >= 2
                ):
                    waits = list(si.on_wait)
                    for k, w in enumerate(waits[:-1]):
                        nop = bass_rust.InstNoOp(name=f"{inst.name}_hoistw{k}")
                        nop.engine = inst.engine
                        nop.sync_info = bass_rust.SyncInfo(
                            on_wait=[w], on_update=[]
                        )
                        out.append(nop)
                    si.on_wait = [waits[-1]]
                    changed = True
                out.append(inst)
            if changed:
                blk.instructions = out


def _build_program():
    import concourse.bass as bass
    import concourse.mybir as mybir
    import concourse.tile as tile

    nc = bass.Bass("TRN2", target_bir_lowering=False, debug=False, num_devices=NCORES)
    xT = nc.dram_tensor("xT", [D, T], mybir.dt.bfloat16, kind="ExternalInput").ap()
    w6 = nc.dram_tensor("w6", [D, 6 * HS], mybir.dt.bfloat16, kind="ExternalInput").ap()
    out = nc.dram_tensor("out", [T, HPC * HS], mybir.dt.float32, kind="ExternalOutput").ap()

    with tile.TileContext(nc) as tc:
        _emit(tc, nc, xT, w6, out)
    _split_matmul_waits(nc)
    return nc


def _in_maps(x, Wk, Wq, Wv):
    import ml_dtypes

    bf = ml_dtypes.bfloat16
    xTh = np.ascontiguousarray(np.asarray(x, dtype=np.float32).T.astype(bf))
    maps = []
    for c in range(NCORES):
        h0, h1 = HPC * c, HPC * c + 1
        W6 = np.concatenate(
            [Wq[h0], Wq[h1], Wk[h0], Wk[h1], Wv[h0], Wv[h1]], axis=1
        ).astype(bf)
        maps.append({"xT": xTh, "w6": np.ascontiguousarray(W6)})
    return maps


def get_program():
    global _cached_nc
    if _cached_nc is None:
        _cached_nc = _build_program()
    return _cached_nc


def kernel(x, Wk, Wq, Wv):
    from concourse.bass_utils import run_bass_kernel_spmd

    nc = get_program()
    res = run_bass_kernel_spmd(nc, _in_maps(x, Wk, Wq, Wv), core_ids=list(range(NCORES)))
    outs = [res.results[c]["out"] for c in range(NCORES)]
    return np.concatenate(outs, axis=1)


# revision 15
# speedup vs baseline: 1.1904x; 1.0386x over previous
"""Multi-head causal attention (scores = K @ Q^T variant) on 8 TRN2 NeuronCores.

Head-parallel sharding: core c computes heads (2c, 2c+1) end-to-end and the
host concatenates the per-core [T, 128] outputs along the feature axis.

Per-core kernel layout notes:
  - Host passes x transposed ([D, T]) and pre-cast to bf16 so every
    projection matmul has the contraction dim (d) on SBUF partitions with
    zero on-device transposes.
  - Q^T/K^T are stored [128, T] with head0 on partitions 0-63 and head1 on
    64-127, letting the S^T matmuls for both heads run concurrently on
    disjoint PE row-groups (tile_position).
  - Scores are computed transposed (S^T[j, i] = Q_j . K_i) so that the AV
    contraction (over j) lands on the partition axis with no transposes.
  - V is stored in natural layout with a fused ones-column ([V | 1]) so a
    single AV matmul produces both the weighted sum and the softmax
    denominator (PSUM row 64).
  - Softmax skips the max-subtraction (scores are ~N(0,1); exp is safe in
    fp32) which matches jax.nn.softmax up to rounding.
  - Matmul operands are bf16 (1 cycle/row on the PE; fp32 streams at ~2);
    PSUM accumulation and the final normalization stay fp32.
  - Projection chunk k and attention i-block k are emitted interleaved
    (i-block k only needs x columns < 512*(k+1)) from one pool set sized to
    exactly 8 PSUM banks, so projections and attention overlap instead of
    serializing on a pool boundary.
"""

import numpy as np

T, D, H, HS = 4096, 1024, 16, 64
NCORES = 8
HPC = H // NCORES  # heads per core = 2
DC = D // 128      # 8 contraction chunks
TC = T // 512      # 8 t-chunks for projections
IB = T // 512      # 8 i-blocks (512 output rows each)
JBN = T // 128     # 32 j-blocks (128 keys each)

_cached_nc = None


def _emit(tc, nc, xT, w6, out):
    import concourse.bass as bass  # noqa: F401
    import concourse.mybir as mybir

    f32 = mybir.dt.float32
    bf16 = mybir.dt.bfloat16
    Exp = mybir.ActivationFunctionType.Exp
    ne = mybir.AluOpType.not_equal
    ge = mybir.AluOpType.is_ge

    with (
        tc.tile_pool(name="const", bufs=1) as constp,
        tc.tile_pool(name="wpool", bufs=1) as wpool,
        tc.tile_pool(name="bigp", bufs=1) as bigp,
        tc.tile_pool(name="xpool", bufs=2) as xpool,
        tc.tile_pool(name="vtp", bufs=2) as vtp,
        tc.tile_pool(name="esp", bufs=4) as esp,
        tc.tile_pool(name="osbp", bufs=2) as osbp,
        tc.tile_pool(name="finp", bufs=4) as finp,
        # PSUM budget (8 banks total): s 2x2 + o 2x1 + p 1 + t 1.
        tc.tile_pool(name="sp", bufs=2, space="PSUM") as sp,
        tc.tile_pool(name="op", bufs=2, space="PSUM") as op,
        tc.tile_pool(name="pp", bufs=1, space="PSUM") as pp,
        tc.tile_pool(name="tp", bufs=1, space="PSUM") as tp,
    ):
        # ---- input DMAs for weights + first x chunk go first ------------
        w6sb = wpool.tile([128, DC, 6 * HS], bf16)
        for dc in range(DC):
            nc.sync.dma_start(out=w6sb[:, dc, :], in_=w6[dc * 128:(dc + 1) * 128, :])
        xts = []
        xt0 = xpool.tile([128, DC, 512], bf16, tag="xt", name="xt0")
        for dc in range(DC):
            nc.sync.dma_start(out=xt0[:, dc, :], in_=xT[dc * 128:(dc + 1) * 128, 0:512])
        xts.append(xt0)

        # ---- constants (gpsimd; overlaps the DMAs) ----------------------
        # id64: two stacked 64x64 identities so both head slices (partition
        # offset 0 and 64) see an identity for the V transposes.
        id64 = constp.tile([128, 64], bf16)
        nc.gpsimd.memset(id64, 0.0)
        nc.gpsimd.affine_select(
            out=id64, in_=id64, compare_op=ne, fill=1.0,
            base=0, channel_multiplier=1, pattern=[[-1, 64]],
        )
        nc.gpsimd.affine_select(
            out=id64, in_=id64, compare_op=ne, fill=1.0,
            base=-64, channel_multiplier=1, pattern=[[-1, 64]],
        )
        # id65: identity for the [65, 128] -> [128, 65] output transposes.
        id65 = constp.tile([128, 65], f32)
        nc.gpsimd.memset(id65, 0.0)
        nc.gpsimd.affine_select(
            out=id65, in_=id65, compare_op=ne, fill=1.0,
            base=0, channel_multiplier=1, pattern=[[-1, 65]],
        )
        # Causal masks for the 4 diagonal block offsets: keep iff il >= jl + 128*q.
        mask4 = constp.tile([128, 4, 512], bf16)
        for q in range(4):
            nc.gpsimd.memset(mask4[:, q, :], 1.0)
            nc.gpsimd.affine_select(
                out=mask4[:, q, :], in_=mask4[:, q, :], compare_op=ge, fill=0.0,
                base=-128 * q, channel_multiplier=-1, pattern=[[1, 512]],
            )

        # ---- persistent activations ------------------------------------
        QT = bigp.tile([128, T], bf16)   # head0 rows 0-63, head1 rows 64-127
        KT = bigp.tile([128, T], bf16)
        Vext0 = bigp.tile([128, JBN, HS + 1], bf16)
        Vext1 = bigp.tile([128, JBN, HS + 1], bf16)
        onesb = constp.tile([128, JBN], bf16)
        nc.gpsimd.memset(onesb, 1.0)
        nc.vector.tensor_copy(Vext0[:, :, HS], onesb)
        nc.vector.tensor_copy(Vext1[:, :, HS], onesb)

        def emit_proj_chunk(tcj):
            ts = slice(tcj * 512, (tcj + 1) * 512)
            if tcj + 1 < TC:  # prefetch next x chunk
                nxt = slice((tcj + 1) * 512, (tcj + 2) * 512)
                xtn = xpool.tile([128, DC, 512], bf16, tag="xt", name=f"xt{tcj + 1}")
                for dc in range(DC):
                    nc.sync.dma_start(
                        out=xtn[:, dc, :], in_=xT[dc * 128:(dc + 1) * 128, nxt]
                    )
                xts.append(xtn)
            xt = xts[tcj]
            for fc, dest in ((0, QT), (1, KT)):
                ps = pp.tile([128, 512], f32, tag="p", name=f"ps{fc}_{tcj}")
                for dc in range(DC):
                    nc.tensor.matmul(
                        ps,
                        lhsT=w6sb[:, dc, fc * 128:(fc + 1) * 128],
                        rhs=xt[:, dc, :],
                        start=(dc == 0), stop=(dc == DC - 1),
                    )
                nc.vector.tensor_copy(dest[:, ts], ps)
            psv = pp.tile([128, 512], f32, tag="p", name=f"psv_{tcj}")
            for dc in range(DC):
                nc.tensor.matmul(
                    psv,
                    lhsT=w6sb[:, dc, 256:384],
                    rhs=xt[:, dc, :],
                    start=(dc == 0), stop=(dc == DC - 1),
                )
            vts = vtp.tile([128, 512], bf16, tag="vts", name=f"vts_{tcj}")
            nc.vector.tensor_copy(vts, psv)
            for h in range(HPC):
                vdst = Vext0 if h == 0 else Vext1
                for q in range(4):
                    ptv = tp.tile([128, 64], bf16, tag="t", name=f"ptv{h}_{tcj}_{q}")
                    nc.tensor.transpose(
                        ptv,
                        in_=vts[h * 64:(h + 1) * 64, q * 128:(q + 1) * 128],
                        identity=id64[h * 64:(h + 1) * 64, :],
                        tile_position=(h * 64, 0),
                    )
                    nc.vector.tensor_copy(vdst[:, tcj * 4 + q, 0:HS], ptv)

        def emit_attn_block(ib):
            isl = slice(ib * 512, (ib + 1) * 512)
            njb = 4 * (ib + 1)
            po = [
                op.tile([65, 512], f32, tag="o", name=f"po{h}_{ib}")
                for h in range(HPC)
            ]
            pending = None
            for jb in range(njb):
                ps = sp.tile([128, 2, 512], f32, tag="s", name=f"s_{ib}_{jb}")
                for h in range(HPC):
                    nc.tensor.matmul(
                        ps[:, h, :],
                        lhsT=QT[h * 64:(h + 1) * 64, jb * 128:(jb + 1) * 128],
                        rhs=KT[h * 64:(h + 1) * 64, isl],
                        start=True, stop=True,
                        tile_position=(h * 64, 0),
                    )
                es = esp.tile([128, 2, 512], bf16, tag="es", name=f"es_{ib}_{jb}")
                nc.scalar.activation(es, ps, Exp, scale=float(1.0 / np.sqrt(HS)))
                q = jb - 4 * ib
                if q >= 0:  # diagonal block: zero out j > i entries
                    for h in range(HPC):
                        nc.vector.tensor_mul(es[:, h, :], es[:, h, :], mask4[:, q, :])
                if pending is not None:
                    pjb, pes = pending
                    for h in range(HPC):
                        nc.tensor.matmul(
                            po[h],
                            lhsT=Vext0[:, pjb, :] if h == 0 else Vext1[:, pjb, :],
                            rhs=pes[:, h, :],
                            start=(pjb == 0), stop=False,
                        )
                pending = (jb, es)
            pjb, pes = pending
            for h in range(HPC):
                nc.tensor.matmul(
                    po[h],
                    lhsT=Vext0[:, pjb, :] if h == 0 else Vext1[:, pjb, :],
                    rhs=pes[:, h, :],
                    start=(pjb == 0), stop=True,
                )
            # finalize: transpose O^T, divide by the denominator row, store
            for h in range(HPC):
                osb = osbp.tile([65, 512], f32, tag="osb", name=f"osb{h}_{ib}")
                nc.vector.tensor_copy(osb, po[h])
                for qq in range(4):
                    pt = tp.tile([128, 65], f32, tag="t", name=f"pt{h}_{ib}_{qq}")
                    nc.tensor.transpose(
                        pt,
                        in_=osb[:, qq * 128:(qq + 1) * 128],
                        identity=id65[0:65, :],
                    )
                    rr = finp.tile([128, 1], f32, tag="rr", name=f"rr{h}_{ib}_{qq}")
                    nc.vector.reciprocal(rr, pt[:, 64:65])
                    ob = finp.tile([128, 64], f32, tag="ob", name=f"ob{h}_{ib}_{qq}")
                    nc.vector.tensor_scalar_mul(ob, pt[:, 0:64], rr)
                    r0 = ib * 512 + qq * 128
                    nc.sync.dma_start(
                        out=out[r0:r0 + 128, h * 64:(h + 1) * 64], in_=ob
                    )

        # Staircase: attention block k only depends on projection chunks <= k.
        for k in range(TC):
            emit_proj_chunk(k)
            emit_attn_block(k)


# walrus engine-instruction encodings have a single sync-wait slot; hoist
# extra waits onto per-wait NoOps for everything except generated NoOps.
_NO_HOIST_TYPES = frozenset({"InstNoOp"})


def _legalize_waits(nc):
    """Hoist multi-waits off engine instructions onto preceding NoOps.

    Most walrus instruction encodings (S3_LW matmul, DMA, ACT, DVE, drain)
    only have room for a single sync-wait command; Tile freely attaches
    several. Waits execute on the engine's sequencer in program order, so
    moving them to immediately-preceding NoOps is semantics-preserving.
    """
    import bass_rust

    for f in nc.m.functions:
        for blk in f.blocks:
            out = []
            changed = False
            for inst in blk.instructions:
                si = getattr(inst, "sync_info", None)
                if (
                    type(inst).__name__ not in _NO_HOIST_TYPES
                    and si is not None
                    and len(si.on_wait) >= 2
                ):
                    waits = list(si.on_wait)
                    for k, w in enumerate(waits[:-1]):
                        nop = bass_rust.InstNoOp(name=f"{inst.name}_hoistw{k}")
                        nop.engine = inst.engine
                        nop.sync_info = bass_rust.SyncInfo(
                            on_wait=[w], on_update=[]
                        )
                        out.append(nop)
                    si.on_wait = [waits[-1]]
                    changed = True
                out.append(inst)
            if changed:
                blk.instructions = out


def _build_program():
    import concourse.bass as bass
    import concourse.mybir as mybir
    import concourse.tile as tile

    nc = bass.Bass("TRN2", target_bir_lowering=False, debug=False, num_devices=NCORES)
    xT = nc.dram_tensor("xT", [D, T], mybir.dt.bfloat16, kind="ExternalInput").ap()
    w6 = nc.dram_tensor("w6", [D, 6 * HS], mybir.dt.bfloat16, kind="ExternalInput").ap()
    out = nc.dram_tensor("out", [T, HPC * HS], mybir.dt.float32, kind="ExternalOutput").ap()

    with tile.TileContext(nc) as tc:
        _emit(tc, nc, xT, w6, out)
    _legalize_waits(nc)
    return nc


def _in_maps(x, Wk, Wq, Wv):
    import ml_dtypes

    bf = ml_dtypes.bfloat16
    xTh = np.ascontiguousarray(np.asarray(x, dtype=np.float32).T.astype(bf))
    maps = []
    for c in range(NCORES):
        h0, h1 = HPC * c, HPC * c + 1
        W6 = np.concatenate(
            [Wq[h0], Wq[h1], Wk[h0], Wk[h1], Wv[h0], Wv[h1]], axis=1
        ).astype(bf)
        maps.append({"xT": xTh, "w6": np.ascontiguousarray(W6)})
    return maps


def get_program():
    global _cached_nc
    if _cached_nc is None:
        _cached_nc = _build_program()
    return _cached_nc


def kernel(x, Wk, Wq, Wv):
    from concourse.bass_utils import run_bass_kernel_spmd

    nc = get_program()
    res = run_bass_kernel_spmd(nc, _in_maps(x, Wk, Wq, Wv), core_ids=list(range(NCORES)))
    outs = [res.results[c]["out"] for c in range(NCORES)]
    return np.concatenate(outs, axis=1)


# revision 17
# speedup vs baseline: 1.3028x; 1.0944x over previous
"""Multi-head causal attention (scores = K @ Q^T variant) on 8 TRN2 NeuronCores.

Head-parallel sharding: core c computes heads (2c, 2c+1) end-to-end and the
host concatenates the per-core [T, 128] outputs along the feature axis.

Per-core kernel layout notes:
  - Host passes x transposed ([D, T]) and pre-cast to bf16 so every
    projection matmul has the contraction dim (d) on SBUF partitions with
    zero on-device transposes.
  - Q^T/K^T are stored [128, T] with head0 on partitions 0-63 and head1 on
    64-127, letting the S^T matmuls for both heads run concurrently on
    disjoint PE row-groups (tile_position).
  - Scores are computed transposed (S^T[j, i] = Q_j . K_i) so that the AV
    contraction (over j) lands on the partition axis with no transposes.
  - V is stored in natural layout with a fused ones-column ([V | 1]) so a
    single AV matmul produces both the weighted sum and the softmax
    denominator (PSUM row 64).
  - Softmax skips the max-subtraction (scores are ~N(0,1); exp is safe in
    fp32) which matches jax.nn.softmax up to rounding.
  - Matmul operands are bf16 (1 cycle/row on the PE; fp32 streams at ~2);
    PSUM accumulation and the final normalization stay fp32.
  - Projection chunk k and attention i-block k are emitted interleaved
    (i-block k only needs x columns < 512*(k+1)) from one pool set sized to
    exactly 8 PSUM banks, so projections and attention overlap instead of
    serializing on a pool boundary.
"""

import numpy as np

T, D, H, HS = 4096, 1024, 16, 64
NCORES = 8
HPC = H // NCORES  # heads per core = 2
DC = D // 128      # 8 contraction chunks
TC = T // 512      # 8 t-chunks for projections
IB = T // 512      # 8 i-blocks (512 output rows each)
JBN = T // 128     # 32 j-blocks (128 keys each)

_cached_nc = None


def _emit(tc, nc, xT, w6, out):
    import concourse.bass as bass  # noqa: F401
    import concourse.mybir as mybir

    f32 = mybir.dt.float32
    bf16 = mybir.dt.bfloat16
    Exp = mybir.ActivationFunctionType.Exp
    ne = mybir.AluOpType.not_equal
    ge = mybir.AluOpType.is_ge

    with (
        tc.tile_pool(name="const", bufs=1) as constp,
        tc.tile_pool(name="wpool", bufs=1) as wpool,
        tc.tile_pool(name="bigp", bufs=1) as bigp,
        tc.tile_pool(name="xpool", bufs=2) as xpool,
        tc.tile_pool(name="vtp", bufs=2) as vtp,
        tc.tile_pool(name="esp", bufs=4) as esp,
        tc.tile_pool(name="finp", bufs=4) as finp,
        # PSUM budget (8 banks total): s 2x2 + o 2x1 + p 1 + t 1.
        tc.tile_pool(name="sp", bufs=2, space="PSUM") as sp,
        tc.tile_pool(name="op", bufs=2, space="PSUM") as op,
        tc.tile_pool(name="pp", bufs=1, space="PSUM") as pp,
        tc.tile_pool(name="tp", bufs=1, space="PSUM") as tp,
    ):
        # ---- input DMAs for weights + first x chunk go first ------------
        w6sb = wpool.tile([128, DC, 6 * HS], bf16)
        xts = []
        xt0 = xpool.tile([128, DC, 512], bf16, tag="xt", name="xt0")
        for dc in range(DC):
            nc.sync.dma_start(out=w6sb[:, dc, :], in_=w6[dc * 128:(dc + 1) * 128, :])
            nc.sync.dma_start(out=xt0[:, dc, :], in_=xT[dc * 128:(dc + 1) * 128, 0:512])
        xts.append(xt0)

        # ---- constants (gpsimd; overlaps the DMAs) ----------------------
        # id64: two stacked 64x64 identities so both head slices (partition
        # offset 0 and 64) see an identity for the V transposes.
        id64 = constp.tile([128, 64], bf16)
        nc.gpsimd.memset(id64, 0.0)
        nc.gpsimd.affine_select(
            out=id64, in_=id64, compare_op=ne, fill=1.0,
            base=0, channel_multiplier=1, pattern=[[-1, 64]],
        )
        nc.gpsimd.affine_select(
            out=id64, in_=id64, compare_op=ne, fill=1.0,
            base=-64, channel_multiplier=1, pattern=[[-1, 64]],
        )
        # Causal masks for the 4 diagonal block offsets: keep iff il >= jl + 128*q.
        mask4 = constp.tile([128, 4, 512], bf16)
        for q in range(4):
            nc.gpsimd.memset(mask4[:, q, :], 1.0)
            nc.gpsimd.affine_select(
                out=mask4[:, q, :], in_=mask4[:, q, :], compare_op=ge, fill=0.0,
                base=-128 * q, channel_multiplier=-1, pattern=[[1, 512]],
            )

        # ---- persistent activations ------------------------------------
        QT = bigp.tile([128, T], bf16)   # head0 rows 0-63, head1 rows 64-127
        KT = bigp.tile([128, T], bf16)
        Vext0 = bigp.tile([128, JBN, HS + 1], bf16)
        Vext1 = bigp.tile([128, JBN, HS + 1], bf16)
        onesb = constp.tile([128, JBN], bf16)
        nc.gpsimd.memset(onesb, 1.0)
        nc.vector.tensor_copy(Vext0[:, :, HS], onesb)
        nc.vector.tensor_copy(Vext1[:, :, HS], onesb)

        def emit_proj_chunk(tcj):
            ts = slice(tcj * 512, (tcj + 1) * 512)
            if tcj + 1 < TC:  # prefetch next x chunk
                nxt = slice((tcj + 1) * 512, (tcj + 2) * 512)
                xtn = xpool.tile([128, DC, 512], bf16, tag="xt", name=f"xt{tcj + 1}")
                for dc in range(DC):
                    nc.sync.dma_start(
                        out=xtn[:, dc, :], in_=xT[dc * 128:(dc + 1) * 128, nxt]
                    )
                xts.append(xtn)
            xt = xts[tcj]
            for fc, dest in ((0, QT), (1, KT)):
                ps = pp.tile([128, 512], f32, tag="p", name=f"ps{fc}_{tcj}")
                for dc in range(DC):
                    nc.tensor.matmul(
                        ps,
                        lhsT=w6sb[:, dc, fc * 128:(fc + 1) * 128],
                        rhs=xt[:, dc, :],
                        start=(dc == 0), stop=(dc == DC - 1),
                    )
                nc.vector.tensor_copy(dest[:, ts], ps)
            psv = pp.tile([128, 512], f32, tag="p", name=f"psv_{tcj}")
            for dc in range(DC):
                nc.tensor.matmul(
                    psv,
                    lhsT=w6sb[:, dc, 256:384],
                    rhs=xt[:, dc, :],
                    start=(dc == 0), stop=(dc == DC - 1),
                )
            vts = vtp.tile([128, 512], bf16, tag="vts", name=f"vts_{tcj}")
            nc.vector.tensor_copy(vts, psv)
            for h in range(HPC):
                vdst = Vext0 if h == 0 else Vext1
                for q in range(4):
                    ptv = tp.tile([128, 64], bf16, tag="t", name=f"ptv{h}_{tcj}_{q}")
                    nc.tensor.transpose(
                        ptv,
                        in_=vts[h * 64:(h + 1) * 64, q * 128:(q + 1) * 128],
                        identity=id64[h * 64:(h + 1) * 64, :],
                        tile_position=(h * 64, 0),
                    )
                    nc.vector.tensor_copy(vdst[:, tcj * 4 + q, 0:HS], ptv)

        def emit_attn_block(ib):
            isl = slice(ib * 512, (ib + 1) * 512)
            njb = 4 * (ib + 1)
            po = [
                op.tile([65, 512], f32, tag="o", name=f"po{h}_{ib}")
                for h in range(HPC)
            ]
            pending = None
            for jb in range(njb):
                ps = sp.tile([128, 2, 512], f32, tag="s", name=f"s_{ib}_{jb}")
                for h in range(HPC):
                    nc.tensor.matmul(
                        ps[:, h, :],
                        lhsT=QT[h * 64:(h + 1) * 64, jb * 128:(jb + 1) * 128],
                        rhs=KT[h * 64:(h + 1) * 64, isl],
                        start=True, stop=True,
                        tile_position=(h * 64, 0),
                    )
                es = esp.tile([128, 2, 512], bf16, tag="es", name=f"es_{ib}_{jb}")
                nc.scalar.activation(es, ps, Exp, scale=float(1.0 / np.sqrt(HS)))
                q = jb - 4 * ib
                if q >= 0:  # diagonal block: zero out j > i entries
                    for h in range(HPC):
                        nc.vector.tensor_mul(es[:, h, :], es[:, h, :], mask4[:, q, :])
                if pending is not None:
                    pjb, pes = pending
                    for h in range(HPC):
                        nc.tensor.matmul(
                            po[h],
                            lhsT=Vext0[:, pjb, :] if h == 0 else Vext1[:, pjb, :],
                            rhs=pes[:, h, :],
                            start=(pjb == 0), stop=False,
                        )
                pending = (jb, es)
            pjb, pes = pending
            for h in range(HPC):
                nc.tensor.matmul(
                    po[h],
                    lhsT=Vext0[:, pjb, :] if h == 0 else Vext1[:, pjb, :],
                    rhs=pes[:, h, :],
                    start=(pjb == 0), stop=True,
                )
            # evacuate the unnormalized O^T + denominator row; the host does
            # the (tiny) divide and the un-transpose during the gather.
            for h in range(HPC):
                ot = finp.tile([65, 512], f32, tag="ot", name=f"ot{h}_{ib}")
                nc.vector.tensor_copy(ot, po[h])
                nc.sync.dma_start(out=out[h * 65:(h + 1) * 65, isl], in_=ot)

        # Staircase: attention block k only depends on projection chunks <= k.
        for k in range(TC):
            emit_proj_chunk(k)
            emit_attn_block(k)


# walrus engine-instruction encodings have a single sync-wait slot; hoist
# extra waits onto per-wait NoOps for everything except generated NoOps.
_NO_HOIST_TYPES = frozenset({"InstNoOp"})


def _legalize_waits(nc):
    """Hoist multi-waits off engine instructions onto preceding NoOps.

    Most walrus instruction encodings (S3_LW matmul, DMA, ACT, DVE, drain)
    only have room for a single sync-wait command; Tile freely attaches
    several. Waits execute on the engine's sequencer in program order, so
    moving them to immediately-preceding NoOps is semantics-preserving.
    """
    import bass_rust

    for f in nc.m.functions:
        for blk in f.blocks:
            out = []
            changed = False
            for inst in blk.instructions:
                si = getattr(inst, "sync_info", None)
                if (
                    type(inst).__name__ not in _NO_HOIST_TYPES
                    and si is not None
                    and len(si.on_wait) >= 2
                ):
                    waits = list(si.on_wait)
                    for k, w in enumerate(waits[:-1]):
                        nop = bass_rust.InstNoOp(name=f"{inst.name}_hoistw{k}")
                        nop.engine = inst.engine
                        nop.sync_info = bass_rust.SyncInfo(
                            on_wait=[w], on_update=[]
                        )
                        out.append(nop)
                    si.on_wait = [waits[-1]]
                    changed = True
                out.append(inst)
            if changed:
                blk.instructions = out


def _build_program():
    import concourse.bass as bass
    import concourse.mybir as mybir
    import concourse.tile as tile

    nc = bass.Bass("TRN2", target_bir_lowering=False, debug=False, num_devices=NCORES)
    xT = nc.dram_tensor("xT", [D, T], mybir.dt.bfloat16, kind="ExternalInput").ap()
    w6 = nc.dram_tensor("w6", [D, 6 * HS], mybir.dt.bfloat16, kind="ExternalInput").ap()
    out = nc.dram_tensor("outR", [HPC * (HS + 1), T], mybir.dt.float32, kind="ExternalOutput").ap()

    with tile.TileContext(nc) as tc:
        _emit(tc, nc, xT, w6, out)
    _legalize_waits(nc)
    return nc


def _in_maps(x, Wk, Wq, Wv):
    import ml_dtypes

    bf = ml_dtypes.bfloat16
    xTh = np.ascontiguousarray(np.asarray(x, dtype=np.float32).T.astype(bf))
    maps = []
    for c in range(NCORES):
        h0, h1 = HPC * c, HPC * c + 1
        W6 = np.concatenate(
            [Wq[h0], Wq[h1], Wk[h0], Wk[h1], Wv[h0], Wv[h1]], axis=1
        ).astype(bf)
        maps.append({"xT": xTh, "w6": np.ascontiguousarray(W6)})
    return maps


def get_program():
    global _cached_nc
    if _cached_nc is None:
        _cached_nc = _build_program()
    return _cached_nc


def kernel(x, Wk, Wq, Wv):
    from concourse.bass_utils import run_bass_kernel_spmd

    nc = get_program()
    res = run_bass_kernel_spmd(nc, _in_maps(x, Wk, Wq, Wv), core_ids=list(range(NCORES)))
    cols = []
    for c in range(NCORES):
        raw = res.results[c]["outR"]  # [2*65, T]: per head 64 rows O^T + denom
        for h in range(HPC):
            o = raw[h * 65:h * 65 + HS]
            den = raw[h * 65 + HS:h * 65 + HS + 1]
            cols.append((o / den).T)
    return np.ascontiguousarray(np.concatenate(cols, axis=1), dtype=np.float32)
